# revision 5
# baseline (speedup 1.0000x reference)
"""Multi-head causal attention (with faithful reference bugs) on 8 TRN2 cores.

Reference semantics (B=4, T=2048, D=1024, H=16, hd=64):
    q = (x @ Wq.T) viewed (B,T,H,hd) -> (B,H,T,hd); same k, v
    scores = (q @ k.T) * sqrt(D)            # "bug": / D**-0.5
    causal mask, softmax
    out = attn @ v                          # (B,H,T,hd)
    att = out.reshape(B, T, H*hd)           # "bug": no transpose back
    y = att @ Wo.T

Because of the reshape bug, output rows group by head: rows
[128h, 128h+128) of y[b] depend only on head h (att row 128h+i is
out[b,h,16i:16i+16,:].reshape(1024)).  Sharding: 8 cores = (batch b,
head-group g) with g in {0,1} covering heads 8g..8g+7; each core
computes y[b, 1024g:1024g+1024, :] fully independently (no collectives).

Per-core pipeline (all matmuls full-rate on the PE):
  Phase 1: QKV projections in fp32r from host-transposed x^T and W^T.
           q^T,k^T spilled to DRAM (x32 folded into q); v kept in SBUF
           as fp16 [t-tile, head, 65] with a ones column for the
           softmax denominator.
  Phase 2 per head:
    prepass: s = 32*q.k in s-layout [q,k] + -1e9 causal mask (PE),
             row max via DVE -> m̂, negated, PE-transposed to a row.
    main:    s̃^T[k,q] = 32*q.k - m̂[q] via a 65-partition matmul
             (extra contraction row carries ones x -m̂), -1e9 diag
             masks via constant step-matrix matmuls, exp on ACT into
             fp16 w̃, A.V as v̂^T @ w̃ with the ones column emitting the
             denominator D per q for free.
    norm:    1/D via DVE reciprocal (partition-major via DRAM bounce),
             DMA partition-broadcast, fused into the PSUM->SBUF scatter
             that undoes the reshape bug (T layout), then y = T @ Wo^T.
"""

import numpy as np
import ml_dtypes

B, T, D, H = 4, 2048, 1024, 16
HD = D // H  # 64
HL = H // 2  # heads per core = 8
SCALE = float(np.sqrt(D))  # 32.0
NEG = -1.0e9

_CACHE = {}


def _build():
    import concourse.bacc as bacc
    import concourse.mybir as mybir
    import concourse.tile as tile
    from concourse.masks import make_identity

    dt = mybir.dt
    f32, f32r, f16, bf16 = dt.float32, dt.float32r, dt.float16, dt.bfloat16
    Exp = mybir.ActivationFunctionType.Exp
    AX = mybir.AxisListType.X

    nc = bacc.Bacc("TRN2", target_bir_lowering=False, debug=False, num_devices=8)

    # ---- DRAM I/O ----
    xT = nc.dram_tensor("xT", [D, T], f32r, kind="ExternalInput")  # x[b].T
    wqT = nc.dram_tensor("wqT", [D, 512], f32r, kind="ExternalInput")  # Wq[g].T
    wkT = nc.dram_tensor("wkT", [D, 512], f32r, kind="ExternalInput")
    wvT = nc.dram_tensor("wvT", [D, 512], f32r, kind="ExternalInput")
    woT = nc.dram_tensor("woT", [D, D], f16, kind="ExternalInput")  # Wo.T fp16
    negI = nc.dram_tensor("negI", [128, 128], bf16, kind="ExternalInput")
    vmask = nc.dram_tensor("vmask", [128, 640], bf16, kind="ExternalInput")
    vmaskP = nc.dram_tensor("vmaskP", [128, 128], bf16, kind="ExternalInput")
    out = nc.dram_tensor("out", [1024, D], f32, kind="ExternalOutput")

    # internal spills
    qsp = nc.dram_tensor("qsp", [4, 128, 4, 512], f32r)  # [ot, p, tb, o] 32*q^T
    ksp = nc.dram_tensor("ksp", [4, 128, 4, 512], f32r)

    with tile.TileContext(nc) as tc:
        with (
            tc.tile_pool(name="const", bufs=1) as cpool,
            tc.tile_pool(name="vres", bufs=1) as vpool,
        ):
            # ---- constants / resident tensors ----
            negI_t = cpool.tile([128, 128], bf16)
            nc.sync.dma_start(negI_t[:], negI[:])
            vmask_t = cpool.tile([128, 640], bf16)
            nc.sync.dma_start(vmask_t[:], vmask[:])
            vmaskP_t = cpool.tile([128, 128], bf16)
            nc.sync.dma_start(vmaskP_t[:], vmaskP[:])
            ident = cpool.tile([128, 128], f32)
            make_identity(nc, ident[:])
            wo_sb = cpool.tile([128, 8, 1024], f16)
            nc.sync.dma_start(wo_sb[:], woT.rearrange("(a p) m -> p a m", p=128))
            # v resident: [p, ttile, head, 65] fp16, col 64 = ones
            v_sb = vpool.tile([128, 16, HL, 65], f16)
            nc.gpsimd.memset(v_sb[:, :, :, 64:65], 1.0)

            # ================= Phase 1: projections =================
            with (
                tc.tile_pool(name="wgt", bufs=1) as wpool,
                tc.tile_pool(name="xch", bufs=16) as xpool,
                tc.tile_pool(name="pstage", bufs=4) as spool,
                tc.tile_pool(name="proj_ps", bufs=4, space="PSUM") as ppool,
            ):
                wq_sb = wpool.tile([128, 8, 512], f32r, tag="w")
                wk_sb = wpool.tile([128, 8, 512], f32r, tag="wk")
                wv_sb = wpool.tile([128, 8, 512], f32r, tag="wv")
                nc.sync.dma_start(wq_sb[:], wqT.rearrange("(a p) m -> p a m", p=128))
                nc.sync.dma_start(wk_sb[:], wkT.rearrange("(a p) m -> p a m", p=128))
                nc.sync.dma_start(wv_sb[:], wvT.rearrange("(a p) m -> p a m", p=128))

                xTr = xT.rearrange("(a p) m -> p a m", p=128)
                for tb in range(4):
                    xch = []
                    for dc in range(8):
                        xc = xpool.tile([128, 512], f32r, tag="x")
                        nc.sync.dma_start(
                            xc[:], xTr[:, dc, tb * 512 : (tb + 1) * 512]
                        )
                        xch.append(xc)
                    # q, k projections -> transposed layout [o, t]
                    for w_sb, sp, scl in ((wq_sb, qsp, SCALE), (wk_sb, ksp, 1.0)):
                        for ot in range(4):
                            ps = ppool.tile([128, 512], f32, tag="ps")
                            for dc in range(8):
                                nc.tensor.matmul(
                                    ps[:],
                                    w_sb[:, dc, ot * 128 : (ot + 1) * 128],
                                    xch[dc][:],
                                    start=(dc == 0),
                                    stop=(dc == 7),
                                )
                            st = spool.tile([128, 512], f32r, tag="st")
                            if scl == 1.0:
                                nc.vector.tensor_copy(st[:], ps[:])
                            else:
                                nc.vector.tensor_scalar_mul(st[:], ps[:], scl)
                            nc.sync.dma_start(sp[ot, :, tb, :], st[:])
                    # v projection -> natural layout [t, o] fp16 into v_sb
                    for tt in range(4):
                        ttile = tb * 4 + tt
                        ps = ppool.tile([128, 512], f32, tag="ps")
                        for dc in range(8):
                            nc.tensor.matmul(
                                ps[:],
                                xch[dc][:, tt * 128 : (tt + 1) * 128],
                                wv_sb[:, dc, :],
                                start=(dc == 0),
                                stop=(dc == 7),
                            )
                        nc.vector.tensor_copy(v_sb[:, ttile, :, 0:64], ps[:])

            # ================= Phase 2: attention =================
            with (
                tc.tile_pool(name="qk", bufs=2) as qkpool,
                tc.tile_pool(name="stat", bufs=2) as mpool,
                tc.tile_pool(name="wexp", bufs=3) as wpoolx,
                tc.tile_pool(name="tt", bufs=2) as tpool,
                tc.tile_pool(name="sm", bufs=3) as smpool,
                tc.tile_pool(name="pre_ps", bufs=1, space="PSUM") as prepool,
                tc.tile_pool(name="s_ps", bufs=2, space="PSUM") as sgpool,
                tc.tile_pool(name="av_ps", bufs=2, space="PSUM") as avpool,
            ):
                for h in range(HL):
                    ot, half = h // 2, h % 2
                    qh = qkpool.tile([65, T], f32r, tag="qh")
                    kh = qkpool.tile([65, T], f32r, tag="kh")
                    nc.sync.dma_start(
                        qh[0:64, :],
                        qsp[ot, half * 64 : half * 64 + 64, :, :],
                    )
                    nc.sync.dma_start(
                        kh[0:64, :],
                        ksp[ot, half * 64 : half * 64 + 64, :, :],
                    )
                    nc.gpsimd.memset(kh[64:65, :].bitcast(f32), 1.0)

                    # ---- prepass: row max -> -m̂ ----
                    mh = mpool.tile([128, 16], f32, tag="mh")
                    for qi in range(16):
                        kext = 128 * (qi + 1)
                        nchk = (kext + 1023) // 1024
                        cmx = mpool.tile([128, 2], f32, tag="cmx")
                        for ch in range(nchk):
                            c0 = ch * 1024
                            cw = min(1024, kext - c0)
                            pre = prepool.tile([128, 1024], f32, tag="pre")
                            for n0 in range(0, cw, 512):
                                nw = min(512, cw - n0)
                                nc.tensor.matmul(
                                    pre[:, n0 : n0 + nw],
                                    qh[0:64, qi * 128 : (qi + 1) * 128],
                                    kh[0:64, c0 + n0 : c0 + n0 + nw],
                                    start=True,
                                    stop=(qi * 128 < c0 + n0 or qi * 128 >= c0 + n0 + nw),
                                    skip_group_check=True,
                                )
                            if c0 <= qi * 128 < c0 + cw:
                                off = qi * 128 - c0
                                nc.tensor.matmul(
                                    pre[:, off : off + 128],
                                    negI_t[:],
                                    vmaskP_t[:],
                                    start=False,
                                    stop=True,
                                    skip_group_check=True,
                                )
                            nc.vector.reduce_max(
                                cmx[:, ch : ch + 1], pre[:, 0:cw], axis=AX
                            )
                        if nchk == 1:
                            nc.vector.tensor_scalar_mul(
                                mh[:, qi : qi + 1], cmx[:, 0:1], -1.0
                            )
                        else:
                            nc.vector.reduce_max(
                                mh[:, qi : qi + 1], cmx[:, 0:2], axis=AX, negate=True
                            )
                    # transpose -m̂ [128,16] -> [16,128] -> row of qh
                    mt_ps = prepool.tile([16, 128], f32, tag="pre")
                    nc.tensor.transpose(mt_ps[:], mh[:], ident[:])
                    mts = mpool.tile([16, 128], f32r, tag="mts")
                    nc.vector.tensor_copy(mts[:], mt_ps[:])
                    # [16,128] partition-major == q order -> one 65th row
                    nc.sync.dma_start(qh[64:65, :], mts[:])

                    # ---- main: s̃^T, exp, A.V per q-block ----
                    th = tpool.tile([128, 1024], f16, tag="th")
                    for qb in range(4):
                        av = avpool.tile([128, 512], f32, tag="av")
                        nkb = 4 * (qb + 1)
                        for kg in range(nkb // 2):
                            sg = sgpool.tile([128, 1024], f32, tag="sg")
                            wsb = wpoolx.tile([128, 1024], f16, tag="wsb")
                            for kk in range(2):
                                kb = kg * 2 + kk
                                j = kb - 4 * qb
                                nc.tensor.matmul(
                                    sg[:, kk * 512 : (kk + 1) * 512],
                                    kh[:, kb * 128 : (kb + 1) * 128],
                                    qh[:, qb * 512 : (qb + 1) * 512],
                                    start=True,
                                    stop=(j < 0),
                                    skip_group_check=True,
                                )
                                if j >= 0:
                                    nc.tensor.matmul(
                                        sg[
                                            :,
                                            kk * 512 : kk * 512 + 128 * (j + 1),
                                        ],
                                        negI_t[:],
                                        vmask_t[
                                            :,
                                            512 - 128 * j : 512 + 128,
                                        ],
                                        start=False,
                                        stop=True,
                                        skip_group_check=True,
                                    )
                            nc.scalar.activation(wsb[:], sg[:], Exp)
                            for kk in range(2):
                                kb = kg * 2 + kk
                                nc.tensor.matmul(
                                    av[0:65, :],
                                    v_sb[:, kb, h, :],
                                    wsb[:, kk * 512 : (kk + 1) * 512],
                                    start=(kb == 0),
                                    stop=(kb == nkb - 1),
                                    skip_group_check=True,
                                )
                        # denominator -> 1/D (partition-major for cheap recip)
                        df = smpool.tile([1, 512], f32, tag="df")
                        nc.scalar.copy(df[:], av[64:65, :])
                        dpm = smpool.tile([128, 4], f32, tag="dpm")
                        nc.sync.dma_start(dpm[:], df[:])  # q = 4p+f
                        rpm = smpool.tile([128, 4], f32, tag="rpm")
                        nc.vector.reciprocal(rpm[:], dpm[:])
                        rf = smpool.tile([1, 512], f32, tag="rf")
                        nc.sync.dma_start(rf[:], rpm[:])
                        bc = smpool.tile([64, 512], f32, tag="bc")
                        nc.gpsimd.partition_broadcast(bc[:], rf[:])
                        # scatter-mul: th[jpar*64+d, jj*128+i] =
                        #   av[d, 16i+2jj+jpar] * bc[...]
                        for jpar in range(2):
                            src = (
                                av[0:64, jpar:512:2]
                                .rearrange("p (i jj) -> p jj i", jj=8)
                            )
                            scl = (
                                bc[:, jpar:512:2]
                                .rearrange("p (i jj) -> p jj i", jj=8)
                            )
                            dst = th[
                                jpar * 64 : jpar * 64 + 64, :
                            ].rearrange("p (jj i4) -> p jj i4", jj=8)[
                                :, :, qb * 32 : (qb + 1) * 32
                            ]
                            nc.vector.tensor_mul(dst, src, scl)
                    # ---- output projection ----
                    for cb in range(2):
                        fo = avpool.tile([128, 512], f32, tag="av")
                        for jj in range(8):
                            nc.tensor.matmul(
                                fo[:],
                                th[:, jj * 128 : (jj + 1) * 128],
                                wo_sb[:, jj, cb * 512 : (cb + 1) * 512],
                                start=(jj == 0),
                                stop=(jj == 7),
                            )
                        fs = smpool.tile([128, 512], f32, tag="fs")
                        nc.vector.tensor_copy(fs[:], fo[:])
                        nc.sync.dma_start(
                            out[h * 128 : (h + 1) * 128, cb * 512 : (cb + 1) * 512],
                            fs[:],
                        )
    nc.compile()
    return nc


def _consts():
    p = np.arange(128)[:, None]
    c = np.arange(640)[None, :]
    vmask = (c < p + 512).astype(ml_dtypes.bfloat16)
    cP = np.arange(128)[None, :]
    vmaskP = (cP > p).astype(ml_dtypes.bfloat16)
    negI = (NEG * np.eye(128)).astype(ml_dtypes.bfloat16)
    return negI, vmask, vmaskP


def kernel(x, Wq, Wk, Wv, Wo):
    x = np.asarray(x, dtype=np.float32)
    Wq = np.asarray(Wq, dtype=np.float32)
    Wk = np.asarray(Wk, dtype=np.float32)
    Wv = np.asarray(Wv, dtype=np.float32)
    Wo = np.asarray(Wo, dtype=np.float32)

    if "nc" not in _CACHE:
        _CACHE["nc"] = _build()
    nc = _CACHE["nc"]

    from concourse.bass_utils import run_bass_kernel_spmd

    negI, vmask, vmaskP = _consts()
    woT = np.ascontiguousarray(Wo.T).astype(np.float16)
    in_maps = []
    for c in range(8):
        b, g = c // 2, c % 2
        sl = slice(512 * g, 512 * (g + 1))
        in_maps.append(
            {
                "xT": np.ascontiguousarray(x[b].T),
                "wqT": np.ascontiguousarray(Wq[sl, :].T),
                "wkT": np.ascontiguousarray(Wk[sl, :].T),
                "wvT": np.ascontiguousarray(Wv[sl, :].T),
                "woT": woT,
                "negI": negI,
                "vmask": vmask,
                "vmaskP": vmaskP,
            }
        )
    res = run_bass_kernel_spmd(nc, in_maps, list(range(8)))
    _CACHE["last_result"] = res
    y = np.empty((B, T, D), dtype=np.float32)
    for c in range(8):
        b, g = c // 2, c % 2
        y[b, 1024 * g : 1024 * (g + 1), :] = res.results[c]["out"]
    return y


# revision 8
# speedup vs baseline: 1.2123x; 1.2123x over previous
"""Multi-head causal attention (with faithful reference bugs) on 8 TRN2 cores.

Reference semantics (B=4, T=2048, D=1024, H=16, hd=64):
    q = (x @ Wq.T) viewed (B,T,H,hd) -> (B,H,T,hd); same k, v
    scores = (q @ k.T) * sqrt(D)            # "bug": / D**-0.5
    causal mask, softmax
    out = attn @ v                          # (B,H,T,hd)
    att = out.reshape(B, T, H*hd)           # "bug": no transpose back
    y = att @ Wo.T

Because of the reshape bug, output rows group by head: rows
[128h, 128h+128) of y[b] depend only on head h (att row 128h+i is
out[b,h,16i:16i+16,:].reshape(1024)).  Sharding: 8 cores = (batch b,
head-group g) with g in {0,1} covering heads 8g..8g+7; each core
computes y[b, 1024g:1024g+1024, :] fully independently (no collectives).

Per-core pipeline (all matmuls full-rate on the PE):
  Phase 1: QKV projections in fp32r from host-transposed x^T and W^T.
           q^T,k^T spilled to DRAM (x32 folded into q); v kept in SBUF
           as fp16 [t-tile, head, 65] with a ones column for the
           softmax denominator.
  Phase 2 per head:
    prepass: s = 32*q.k in s-layout [q,k] + -1e9 causal mask (PE),
             row max via DVE -> m̂, negated, PE-transposed to a row.
    main:    s̃^T[k,q] = 32*q.k - m̂[q] via a 65-partition matmul
             (extra contraction row carries ones x -m̂), -1e9 diag
             masks via constant step-matrix matmuls, exp on ACT into
             fp16 w̃, A.V as v̂^T @ w̃ with the ones column emitting the
             denominator D per q for free.
    norm:    1/D via DVE reciprocal (partition-major via DRAM bounce),
             DMA partition-broadcast, fused into the PSUM->SBUF scatter
             that undoes the reshape bug (T layout), then y = T @ Wo^T.
"""

import numpy as np
import ml_dtypes

B, T, D, H = 4, 2048, 1024, 16
HD = D // H  # 64
HL = H // 2  # heads per core = 8
SCALE = float(np.sqrt(D))  # 32.0
NEG = -1.0e9

_CACHE = {}


def _build():
    import concourse.bacc as bacc
    import concourse.mybir as mybir
    import concourse.tile as tile
    from concourse.masks import make_identity

    dt = mybir.dt
    f32, f32r, f16, bf16 = dt.float32, dt.float32r, dt.float16, dt.bfloat16
    Exp = mybir.ActivationFunctionType.Exp
    AX = mybir.AxisListType.X

    nc = bacc.Bacc("TRN2", target_bir_lowering=False, debug=False, num_devices=8)

    # ---- DRAM I/O ----
    xT = nc.dram_tensor("xT", [D, T], f16, kind="ExternalInput")  # x[b].T
    wqT = nc.dram_tensor("wqT", [D, 512], f16, kind="ExternalInput")  # Wq[g].T
    wkT = nc.dram_tensor("wkT", [D, 512], f16, kind="ExternalInput")
    wvT = nc.dram_tensor("wvT", [D, 512], f16, kind="ExternalInput")
    woT = nc.dram_tensor("woT", [D, D], f16, kind="ExternalInput")  # Wo.T fp16
    negI = nc.dram_tensor("negI", [128, 128], bf16, kind="ExternalInput")
    vmask = nc.dram_tensor("vmask", [128, 640], bf16, kind="ExternalInput")
    vmaskP = nc.dram_tensor("vmaskP", [128, 128], bf16, kind="ExternalInput")
    out = nc.dram_tensor("out", [1024, D], f32, kind="ExternalOutput")

    # internal spills
    qsp = nc.dram_tensor("qsp", [4, 128, 4, 512], f16)  # [ot, p, tb, o] 32*q^T
    ksp = nc.dram_tensor("ksp", [4, 128, 4, 512], f16)

    with tile.TileContext(nc) as tc:
        with (
            tc.tile_pool(name="const", bufs=1) as cpool,
            tc.tile_pool(name="vres", bufs=1) as vpool,
        ):
            # ---- constants / resident tensors ----
            negI_t = cpool.tile([128, 128], bf16)
            nc.sync.dma_start(negI_t[:], negI[:])
            vmask_t = cpool.tile([128, 640], bf16)
            nc.sync.dma_start(vmask_t[:], vmask[:])
            vmaskP_t = cpool.tile([128, 128], bf16)
            nc.sync.dma_start(vmaskP_t[:], vmaskP[:])
            ident = cpool.tile([128, 128], f32)
            make_identity(nc, ident[:])
            wo_sb = cpool.tile([128, 8, 1024], f16)
            nc.sync.dma_start(wo_sb[:], woT.rearrange("(a p) m -> p a m", p=128))
            # v resident: [p, ttile, head, 65] fp16, col 64 = ones
            v_sb = vpool.tile([128, 16, HL, 65], f16)
            nc.gpsimd.memset(v_sb[:, :, :, 64:65], 1.0)

            # ================= Phase 1: projections =================
            with (
                tc.tile_pool(name="wgt", bufs=1) as wpool,
                tc.tile_pool(name="xch", bufs=16) as xpool,
                tc.tile_pool(name="pstage", bufs=4) as spool,
                tc.tile_pool(name="proj_ps", bufs=4, space="PSUM") as ppool,
            ):
                wq_sb = wpool.tile([128, 8, 512], f16, tag="w")
                wk_sb = wpool.tile([128, 8, 512], f16, tag="wk")
                wv_sb = wpool.tile([128, 8, 512], f16, tag="wv")
                nc.sync.dma_start(wq_sb[:], wqT.rearrange("(a p) m -> p a m", p=128))
                nc.sync.dma_start(wk_sb[:], wkT.rearrange("(a p) m -> p a m", p=128))
                nc.sync.dma_start(wv_sb[:], wvT.rearrange("(a p) m -> p a m", p=128))

                xTr = xT.rearrange("(a p) m -> p a m", p=128)
                for tb in range(4):
                    xch = []
                    for dc in range(8):
                        xc = xpool.tile([128, 512], f16, tag="x")
                        nc.sync.dma_start(
                            xc[:], xTr[:, dc, tb * 512 : (tb + 1) * 512]
                        )
                        xch.append(xc)
                    # q, k projections -> transposed layout [o, t]
                    for w_sb, sp, scl in ((wq_sb, qsp, SCALE), (wk_sb, ksp, 1.0)):
                        for ot in range(4):
                            ps = ppool.tile([128, 512], f32, tag="ps")
                            for dc in range(8):
                                nc.tensor.matmul(
                                    ps[:],
                                    w_sb[:, dc, ot * 128 : (ot + 1) * 128],
                                    xch[dc][:],
                                    start=(dc == 0),
                                    stop=(dc == 7),
                                )
                            st = spool.tile([128, 512], f16, tag="st")
                            if scl == 1.0:
                                nc.vector.tensor_copy(st[:], ps[:])
                            else:
                                nc.vector.tensor_scalar_mul(st[:], ps[:], scl)
                            nc.sync.dma_start(sp[ot, :, tb, :], st[:])
                    # v projection -> natural layout [t, o] fp16 into v_sb
                    for tt in range(4):
                        ttile = tb * 4 + tt
                        ps = ppool.tile([128, 512], f32, tag="ps")
                        for dc in range(8):
                            nc.tensor.matmul(
                                ps[:],
                                xch[dc][:, tt * 128 : (tt + 1) * 128],
                                wv_sb[:, dc, :],
                                start=(dc == 0),
                                stop=(dc == 7),
                            )
                        nc.vector.tensor_copy(v_sb[:, ttile, :, 0:64], ps[:])

            # ================= Phase 2: attention =================
            with (
                tc.tile_pool(name="qk", bufs=2) as qkpool,
                tc.tile_pool(name="stat", bufs=2) as mpool,
                tc.tile_pool(name="wexp", bufs=3) as wpoolx,
                tc.tile_pool(name="tt", bufs=2) as tpool,
                tc.tile_pool(name="sm", bufs=3) as smpool,
                tc.tile_pool(name="pre_ps", bufs=1, space="PSUM") as prepool,
                tc.tile_pool(name="s_ps", bufs=2, space="PSUM") as sgpool,
                tc.tile_pool(name="av_ps", bufs=2, space="PSUM") as avpool,
            ):
                qk_tiles = {}

                def emit_load_prepass(h):
                    ot, half = h // 2, h % 2
                    qh = qkpool.tile([65, T], f16, tag="qh")
                    kh = qkpool.tile([65, T], f16, tag="kh")
                    nc.sync.dma_start(
                        qh[0:64, :], qsp[ot, half * 64 : half * 64 + 64, :, :]
                    )
                    nc.sync.dma_start(
                        kh[0:64, :], ksp[ot, half * 64 : half * 64 + 64, :, :]
                    )
                    nc.gpsimd.memset(kh[64:65, :], 1.0)
                    qk_tiles[h] = (qh, kh)
                    # prepass: row max over causal k -> -m̂ -> 65th row of qh
                    mh = mpool.tile([128, 16], f32, tag="mh")
                    for qi in range(16):
                        kext = 128 * (qi + 1)
                        nchk = (kext + 1023) // 1024
                        cmx = mpool.tile([128, 2], f32, tag="cmx")
                        for ch in range(nchk):
                            c0 = ch * 1024
                            cw = min(1024, kext - c0)
                            pre = prepool.tile([128, 1024], f32, tag="pre")
                            for n0 in range(0, cw, 512):
                                nw = min(512, cw - n0)
                                nc.tensor.matmul(
                                    pre[:, n0 : n0 + nw],
                                    qh[0:64, qi * 128 : (qi + 1) * 128],
                                    kh[0:64, c0 + n0 : c0 + n0 + nw],
                                    start=True,
                                    stop=(
                                        qi * 128 < c0 + n0
                                        or qi * 128 >= c0 + n0 + nw
                                    ),
                                    skip_group_check=True,
                                )
                            if c0 <= qi * 128 < c0 + cw:
                                off = qi * 128 - c0
                                nc.tensor.matmul(
                                    pre[:, off : off + 128],
                                    negI_t[:],
                                    vmaskP_t[:],
                                    start=False,
                                    stop=True,
                                    skip_group_check=True,
                                )
                            nc.vector.reduce_max(
                                cmx[:, ch : ch + 1], pre[:, 0:cw], axis=AX
                            )
                        if nchk == 1:
                            nc.vector.tensor_scalar_mul(
                                mh[:, qi : qi + 1], cmx[:, 0:1], -1.0
                            )
                        else:
                            nc.vector.reduce_max(
                                mh[:, qi : qi + 1],
                                cmx[:, 0:2],
                                axis=AX,
                                negate=True,
                            )
                    mt_ps = prepool.tile([16, 128], f32, tag="pre")
                    nc.tensor.transpose(mt_ps[:], mh[:], ident[:])
                    mts = mpool.tile([16, 128], f16, tag="mts")
                    nc.vector.tensor_copy(mts[:], mt_ps[:])
                    # [16,128] partition-major == q order -> one 65th row
                    nc.sync.dma_start(qh[64:65, :], mts[:])

                def emit_main(h):
                    qh, kh = qk_tiles.pop(h)
                    th = tpool.tile([128, 1024], f16, tag="th")
                    for qb in range(4):
                        av = avpool.tile([128, 512], f32, tag="av")
                        nkb = 4 * (qb + 1)
                        for kg in range(nkb // 2):
                            sg = sgpool.tile([128, 1024], f32, tag="sg")
                            wsb = wpoolx.tile([128, 1024], f16, tag="wsb")
                            for kk in range(2):
                                kb = kg * 2 + kk
                                j = kb - 4 * qb
                                nc.tensor.matmul(
                                    sg[:, kk * 512 : (kk + 1) * 512],
                                    kh[:, kb * 128 : (kb + 1) * 128],
                                    qh[:, qb * 512 : (qb + 1) * 512],
                                    start=True,
                                    stop=(j < 0),
                                    skip_group_check=True,
                                )
                                if j >= 0:
                                    nc.tensor.matmul(
                                        sg[:, kk * 512 : kk * 512 + 128 * (j + 1)],
                                        negI_t[:],
                                        vmask_t[:, 512 - 128 * j : 512 + 128],
                                        start=False,
                                        stop=True,
                                        skip_group_check=True,
                                    )
                            nc.scalar.activation(wsb[:], sg[:], Exp)
                            for kk in range(2):
                                kb = kg * 2 + kk
                                nc.tensor.matmul(
                                    av[0:65, :],
                                    v_sb[:, kb, h, :],
                                    wsb[:, kk * 512 : (kk + 1) * 512],
                                    start=(kb == 0),
                                    stop=(kb == nkb - 1),
                                    skip_group_check=True,
                                )
                        # denominator -> 1/D (partition-major for cheap recip)
                        df = smpool.tile([1, 512], f32, tag="df")
                        nc.scalar.copy(df[:], av[64:65, :])
                        dpm = smpool.tile([128, 4], f32, tag="dpm")
                        nc.sync.dma_start(dpm[:], df[:])  # q = 4p+f
                        rpm = smpool.tile([128, 4], f32, tag="rpm")
                        nc.vector.reciprocal(rpm[:], dpm[:])
                        rf = smpool.tile([1, 512], f32, tag="rf")
                        nc.sync.dma_start(rf[:], rpm[:])
                        bc = smpool.tile([64, 512], f32, tag="bc")
                        nc.gpsimd.partition_broadcast(bc[:], rf[:])
                        # scatter-mul: th[jpar*64+d, jj*128+i] =
                        #   av[d, 16i+2jj+jpar] * bc[...]
                        for jpar in range(2):
                            src = av[0:64, jpar:512:2].rearrange(
                                "p (i jj) -> p jj i", jj=8
                            )
                            scl = bc[:, jpar:512:2].rearrange(
                                "p (i jj) -> p jj i", jj=8
                            )
                            dst = th[
                                jpar * 64 : jpar * 64 + 64, :
                            ].rearrange("p (jj i4) -> p jj i4", jj=8)[
                                :, :, qb * 32 : (qb + 1) * 32
                            ]
                            nc.vector.tensor_mul(dst, src, scl)
                    # output projection
                    for cb in range(2):
                        fo = avpool.tile([128, 512], f32, tag="av")
                        for jj in range(8):
                            nc.tensor.matmul(
                                fo[:],
                                th[:, jj * 128 : (jj + 1) * 128],
                                wo_sb[:, jj, cb * 512 : (cb + 1) * 512],
                                start=(jj == 0),
                                stop=(jj == 7),
                            )
                        fs = smpool.tile([128, 512], f32, tag="fs")
                        nc.vector.tensor_copy(fs[:], fo[:])
                        nc.sync.dma_start(
                            out[
                                h * 128 : (h + 1) * 128,
                                cb * 512 : (cb + 1) * 512,
                            ],
                            fs[:],
                        )

                # software pipeline: prepass(h+1) emitted before main(h) so
                # its PE chunks fill the gaps while DVE reduces run
                emit_load_prepass(0)
                for h in range(HL):
                    if h + 1 < HL:
                        emit_load_prepass(h + 1)
                    emit_main(h)
    nc.compile()
    return nc


def _consts():
    p = np.arange(128)[:, None]
    c = np.arange(640)[None, :]
    vmask = (c < p + 512).astype(ml_dtypes.bfloat16)
    cP = np.arange(128)[None, :]
    vmaskP = (cP > p).astype(ml_dtypes.bfloat16)
    negI = (NEG * np.eye(128)).astype(ml_dtypes.bfloat16)
    return negI, vmask, vmaskP


def kernel(x, Wq, Wk, Wv, Wo):
    x = np.asarray(x, dtype=np.float32)
    Wq = np.asarray(Wq, dtype=np.float32)
    Wk = np.asarray(Wk, dtype=np.float32)
    Wv = np.asarray(Wv, dtype=np.float32)
    Wo = np.asarray(Wo, dtype=np.float32)

    if "nc" not in _CACHE:
        _CACHE["nc"] = _build()
    nc = _CACHE["nc"]

    from concourse.bass_utils import run_bass_kernel_spmd

    negI, vmask, vmaskP = _consts()
    woT = np.ascontiguousarray(Wo.T).astype(np.float16)
    in_maps = []
    for c in range(8):
        b, g = c // 2, c % 2
        sl = slice(512 * g, 512 * (g + 1))
        in_maps.append(
            {
                "xT": np.ascontiguousarray(x[b].T).astype(np.float16),
                "wqT": np.ascontiguousarray(Wq[sl, :].T).astype(np.float16),
                "wkT": np.ascontiguousarray(Wk[sl, :].T).astype(np.float16),
                "wvT": np.ascontiguousarray(Wv[sl, :].T).astype(np.float16),
                "woT": woT,
                "negI": negI,
                "vmask": vmask,
                "vmaskP": vmaskP,
            }
        )
    res = run_bass_kernel_spmd(nc, in_maps, list(range(8)))
    _CACHE["last_result"] = res
    y = np.empty((B, T, D), dtype=np.float32)
    for c in range(8):
        b, g = c // 2, c % 2
        y[b, 1024 * g : 1024 * (g + 1), :] = res.results[c]["out"]
    return y


# revision 9
# speedup vs baseline: 1.2304x; 1.0149x over previous
"""Multi-head causal attention (with faithful reference bugs) on 8 TRN2 cores.

Reference semantics (B=4, T=2048, D=1024, H=16, hd=64):
    q = (x @ Wq.T) viewed (B,T,H,hd) -> (B,H,T,hd); same k, v
    scores = (q @ k.T) * sqrt(D)            # "bug": / D**-0.5
    causal mask, softmax
    out = attn @ v                          # (B,H,T,hd)
    att = out.reshape(B, T, H*hd)           # "bug": no transpose back
    y = att @ Wo.T

Because of the reshape bug, output rows group by head: rows
[128h, 128h+128) of y[b] depend only on head h (att row 128h+i is
out[b,h,16i:16i+16,:].reshape(1024)).  Sharding: 8 cores = (batch b,
head-group g) with g in {0,1} covering heads 8g..8g+7; each core
computes y[b, 1024g:1024g+1024, :] fully independently (no collectives).

Per-core pipeline (all matmuls full-rate on the PE):
  Phase 1: QKV projections in fp32r from host-transposed x^T and W^T.
           q^T,k^T spilled to DRAM (x32 folded into q); v kept in SBUF
           as fp16 [t-tile, head, 65] with a ones column for the
           softmax denominator.
  Phase 2 per head:
    prepass: s = 32*q.k in s-layout [q,k] + -1e9 causal mask (PE),
             row max via DVE -> m̂, negated, PE-transposed to a row.
    main:    s̃^T[k,q] = 32*q.k - m̂[q] via a 65-partition matmul
             (extra contraction row carries ones x -m̂), -1e9 diag
             masks via constant step-matrix matmuls, exp on ACT into
             fp16 w̃, A.V as v̂^T @ w̃ with the ones column emitting the
             denominator D per q for free.
    norm:    1/D via DVE reciprocal (partition-major via DRAM bounce),
             DMA partition-broadcast, fused into the PSUM->SBUF scatter
             that undoes the reshape bug (T layout), then y = T @ Wo^T.
"""

import numpy as np
import ml_dtypes

B, T, D, H = 4, 2048, 1024, 16
HD = D // H  # 64
HL = H // 2  # heads per core = 8
SCALE = float(np.sqrt(D))  # 32.0
NEG = -1.0e9

_CACHE = {}


def _build():
    import concourse.bacc as bacc
    import concourse.mybir as mybir
    import concourse.tile as tile
    from concourse.masks import make_identity

    dt = mybir.dt
    f32, f32r, f16, bf16 = dt.float32, dt.float32r, dt.float16, dt.bfloat16
    Exp = mybir.ActivationFunctionType.Exp
    AX = mybir.AxisListType.X

    nc = bacc.Bacc("TRN2", target_bir_lowering=False, debug=False, num_devices=8)

    # ---- DRAM I/O ----
    xT = nc.dram_tensor("xT", [D, T], f16, kind="ExternalInput")  # x[b].T
    wqT = nc.dram_tensor("wqT", [D, 512], f16, kind="ExternalInput")  # Wq[g].T
    wkT = nc.dram_tensor("wkT", [D, 512], f16, kind="ExternalInput")
    wvT = nc.dram_tensor("wvT", [D, 512], f16, kind="ExternalInput")
    woT = nc.dram_tensor("woT", [D, D], f16, kind="ExternalInput")  # Wo.T fp16
    negI = nc.dram_tensor("negI", [128, 128], bf16, kind="ExternalInput")
    vmask = nc.dram_tensor("vmask", [128, 640], bf16, kind="ExternalInput")
    vmaskP = nc.dram_tensor("vmaskP", [128, 128], bf16, kind="ExternalInput")
    out = nc.dram_tensor("out", [1024, D], f32, kind="ExternalOutput")

    # internal spills
    qsp = nc.dram_tensor("qsp", [4, 128, 4, 512], f16)  # [ot, p, tb, o] 32*q^T
    ksp = nc.dram_tensor("ksp", [4, 128, 4, 512], f16)

    with tile.TileContext(nc) as tc:
        with (
            tc.tile_pool(name="const", bufs=1) as cpool,
            tc.tile_pool(name="vres", bufs=1) as vpool,
        ):
            # ---- constants / resident tensors ----
            negI_t = cpool.tile([128, 128], bf16)
            nc.sync.dma_start(negI_t[:], negI[:])
            vmask_t = cpool.tile([128, 640], bf16)
            nc.sync.dma_start(vmask_t[:], vmask[:])
            vmaskP_t = cpool.tile([128, 128], bf16)
            nc.sync.dma_start(vmaskP_t[:], vmaskP[:])
            ident = cpool.tile([128, 128], f32)
            make_identity(nc, ident[:])
            wo_sb = cpool.tile([128, 8, 1024], f16)
            nc.sync.dma_start(wo_sb[:], woT.rearrange("(a p) m -> p a m", p=128))
            # v resident: [p, ttile, head, 65] fp16, col 64 = ones
            v_sb = vpool.tile([128, 16, HL, 65], f16)
            nc.gpsimd.memset(v_sb[:, :, :, 64:65], 1.0)

            # ================= Phase 1: projections =================
            # otile-outer so each head-pair's q/k spill completes early and
            # its prepass (DVE row-max chains) overlaps remaining projections
            with (
                tc.tile_pool(name="qk", bufs=6) as qkpool,
                tc.tile_pool(name="stat", bufs=3) as mpool,
                tc.tile_pool(name="pre_ps", bufs=1, space="PSUM") as prepool,
            ):
                qk_tiles = {}

                def emit_load_prepass(h):
                    ot, half = h // 2, h % 2
                    qh = qkpool.tile([65, T], f16, tag="qh")
                    kh = qkpool.tile([65, T], f16, tag="kh")
                    nc.sync.dma_start(
                        qh[0:64, :], qsp[ot, half * 64 : half * 64 + 64, :, :]
                    )
                    nc.sync.dma_start(
                        kh[0:64, :], ksp[ot, half * 64 : half * 64 + 64, :, :]
                    )
                    nc.gpsimd.memset(kh[64:65, :], 1.0)
                    qk_tiles[h] = (qh, kh)
                    # prepass: row max over causal k -> -m̂ -> 65th row of qh
                    mh = mpool.tile([128, 16], f32, tag="mh")
                    for qi in range(16):
                        kext = 128 * (qi + 1)
                        nchk = (kext + 1023) // 1024
                        cmx = mpool.tile([128, 2], f32, tag="cmx")
                        for ch in range(nchk):
                            c0 = ch * 1024
                            cw = min(1024, kext - c0)
                            pre = prepool.tile([128, 1024], f32, tag="pre")
                            for n0 in range(0, cw, 512):
                                nw = min(512, cw - n0)
                                nc.tensor.matmul(
                                    pre[:, n0 : n0 + nw],
                                    qh[0:64, qi * 128 : (qi + 1) * 128],
                                    kh[0:64, c0 + n0 : c0 + n0 + nw],
                                    start=True,
                                    stop=(
                                        qi * 128 < c0 + n0
                                        or qi * 128 >= c0 + n0 + nw
                                    ),
                                    skip_group_check=True,
                                )
                            if c0 <= qi * 128 < c0 + cw:
                                off = qi * 128 - c0
                                nc.tensor.matmul(
                                    pre[:, off : off + 128],
                                    negI_t[:],
                                    vmaskP_t[:],
                                    start=False,
                                    stop=True,
                                    skip_group_check=True,
                                )
                            nc.vector.reduce_max(
                                cmx[:, ch : ch + 1], pre[:, 0:cw], axis=AX
                            )
                        if nchk == 1:
                            nc.vector.tensor_scalar_mul(
                                mh[:, qi : qi + 1], cmx[:, 0:1], -1.0
                            )
                        else:
                            nc.vector.reduce_max(
                                mh[:, qi : qi + 1],
                                cmx[:, 0:2],
                                axis=AX,
                                negate=True,
                            )
                    mt_ps = prepool.tile([16, 128], f32, tag="pre")
                    nc.tensor.transpose(mt_ps[:], mh[:], ident[:])
                    mts = mpool.tile([16, 128], f16, tag="mts")
                    nc.vector.tensor_copy(mts[:], mt_ps[:])
                    # [16,128] partition-major == q order -> one 65th row
                    nc.sync.dma_start(qh[64:65, :], mts[:])

                with (
                    tc.tile_pool(name="wgt", bufs=1) as wpool,
                    tc.tile_pool(name="xch", bufs=32) as xpool,
                    tc.tile_pool(name="pstage", bufs=4) as spool,
                    tc.tile_pool(name="proj_ps", bufs=2, space="PSUM") as ppool,
                ):
                    wq_sb = wpool.tile([128, 8, 512], f16, tag="w")
                    wk_sb = wpool.tile([128, 8, 512], f16, tag="wk")
                    wv_sb = wpool.tile([128, 8, 512], f16, tag="wv")
                    nc.sync.dma_start(
                        wq_sb[:], wqT.rearrange("(a p) m -> p a m", p=128)
                    )
                    nc.sync.dma_start(
                        wk_sb[:], wkT.rearrange("(a p) m -> p a m", p=128)
                    )
                    nc.sync.dma_start(
                        wv_sb[:], wvT.rearrange("(a p) m -> p a m", p=128)
                    )

                    xTr = xT.rearrange("(a p) m -> p a m", p=128)
                    xch = {}
                    for dc in range(8):
                        for tb in range(4):
                            xc = xpool.tile([128, 512], f16, tag="x")
                            nc.sync.dma_start(
                                xc[:], xTr[:, dc, tb * 512 : (tb + 1) * 512]
                            )
                            xch[(dc, tb)] = xc

                    for ot in range(4):
                        for w_sb, sp, scl in (
                            (wq_sb, qsp, SCALE),
                            (wk_sb, ksp, 1.0),
                        ):
                            for tb in range(4):
                                ps = ppool.tile([128, 512], f32, tag="ps")
                                for dc in range(8):
                                    nc.tensor.matmul(
                                        ps[:],
                                        w_sb[:, dc, ot * 128 : (ot + 1) * 128],
                                        xch[(dc, tb)][:],
                                        start=(dc == 0),
                                        stop=(dc == 7),
                                    )
                                st = spool.tile([128, 512], f16, tag="st")
                                if scl == 1.0:
                                    nc.vector.tensor_copy(st[:], ps[:])
                                else:
                                    nc.vector.tensor_scalar_mul(st[:], ps[:], scl)
                                nc.sync.dma_start(sp[ot, :, tb, :], st[:])
                        emit_load_prepass(2 * ot)
                        emit_load_prepass(2 * ot + 1)
                        # v projection for this otile's t-range spread
                        for tt in range(4):
                            ttile = ot * 4 + tt
                            tb, tsub = ttile // 4, ttile % 4
                            ps = ppool.tile([128, 512], f32, tag="ps")
                            for dc in range(8):
                                nc.tensor.matmul(
                                    ps[:],
                                    xch[(dc, tb)][:, tsub * 128 : (tsub + 1) * 128],
                                    wv_sb[:, dc, :],
                                    start=(dc == 0),
                                    stop=(dc == 7),
                                )
                            nc.vector.tensor_copy(v_sb[:, ttile, :, 0:64], ps[:])

                # ============== Phase 2: attention mains ==============
                with (
                    tc.tile_pool(name="wexp", bufs=3) as wpoolx,
                    tc.tile_pool(name="tt", bufs=2) as tpool,
                    tc.tile_pool(name="sm", bufs=4) as smpool,
                    tc.tile_pool(name="s_ps", bufs=2, space="PSUM") as sgpool,
                    tc.tile_pool(name="av_ps", bufs=2, space="PSUM") as avpool,
                ):
                    def emit_main(h):
                        qh, kh = qk_tiles.pop(h)
                        th = tpool.tile([128, 1024], f16, tag="th")
                        for qb in range(4):
                            av = avpool.tile([128, 512], f32, tag="av")
                            nkb = 4 * (qb + 1)
                            for kg in range(nkb // 2):
                                sg = sgpool.tile([128, 1024], f32, tag="sg")
                                wsb = wpoolx.tile([128, 1024], f16, tag="wsb")
                                for kk in range(2):
                                    kb = kg * 2 + kk
                                    j = kb - 4 * qb
                                    nc.tensor.matmul(
                                        sg[:, kk * 512 : (kk + 1) * 512],
                                        kh[:, kb * 128 : (kb + 1) * 128],
                                        qh[:, qb * 512 : (qb + 1) * 512],
                                        start=True,
                                        stop=(j < 0),
                                        skip_group_check=True,
                                    )
                                    if j >= 0:
                                        nc.tensor.matmul(
                                            sg[
                                                :,
                                                kk * 512 : kk * 512
                                                + 128 * (j + 1),
                                            ],
                                            negI_t[:],
                                            vmask_t[:, 512 - 128 * j : 512 + 128],
                                            start=False,
                                            stop=True,
                                            skip_group_check=True,
                                        )
                                nc.scalar.activation(wsb[:], sg[:], Exp)
                                for kk in range(2):
                                    kb = kg * 2 + kk
                                    nc.tensor.matmul(
                                        av[0:65, :],
                                        v_sb[:, kb, h, :],
                                        wsb[:, kk * 512 : (kk + 1) * 512],
                                        start=(kb == 0),
                                        stop=(kb == nkb - 1),
                                        skip_group_check=True,
                                    )
                            # denominator -> 1/D (partition-major for recip)
                            df = smpool.tile([1, 512], f32, tag="df")
                            nc.scalar.copy(df[:], av[64:65, :])
                            dpm = smpool.tile([128, 4], f32, tag="dpm")
                            nc.sync.dma_start(dpm[:], df[:])  # q = 4p+f
                            rpm = smpool.tile([128, 4], f32, tag="rpm")
                            nc.vector.reciprocal(rpm[:], dpm[:])
                            rf = smpool.tile([1, 512], f32, tag="rf")
                            nc.sync.dma_start(rf[:], rpm[:])
                            bc = smpool.tile([64, 512], f32, tag="bc")
                            nc.gpsimd.partition_broadcast(bc[:], rf[:])
                            # scatter-mul undoing the reshape bug
                            for jpar in range(2):
                                src = av[0:64, jpar:512:2].rearrange(
                                    "p (i jj) -> p jj i", jj=8
                                )
                                scl = bc[:, jpar:512:2].rearrange(
                                    "p (i jj) -> p jj i", jj=8
                                )
                                dst = th[
                                    jpar * 64 : jpar * 64 + 64, :
                                ].rearrange("p (jj i4) -> p jj i4", jj=8)[
                                    :, :, qb * 32 : (qb + 1) * 32
                                ]
                                nc.vector.tensor_mul(dst, src, scl)
                        # output projection
                        for cb in range(2):
                            fo = avpool.tile([128, 512], f32, tag="av")
                            for jj in range(8):
                                nc.tensor.matmul(
                                    fo[:],
                                    th[:, jj * 128 : (jj + 1) * 128],
                                    wo_sb[:, jj, cb * 512 : (cb + 1) * 512],
                                    start=(jj == 0),
                                    stop=(jj == 7),
                                )
                            fs = smpool.tile([128, 512], f32, tag="fs")
                            nc.vector.tensor_copy(fs[:], fo[:])
                            nc.sync.dma_start(
                                out[
                                    h * 128 : (h + 1) * 128,
                                    cb * 512 : (cb + 1) * 512,
                                ],
                                fs[:],
                            )

                    for h in range(HL):
                        emit_main(h)
    nc.compile()
    return nc


def _consts():
    p = np.arange(128)[:, None]
    c = np.arange(640)[None, :]
    vmask = (c < p + 512).astype(ml_dtypes.bfloat16)
    cP = np.arange(128)[None, :]
    vmaskP = (cP > p).astype(ml_dtypes.bfloat16)
    negI = (NEG * np.eye(128)).astype(ml_dtypes.bfloat16)
    return negI, vmask, vmaskP


def kernel(x, Wq, Wk, Wv, Wo):
    x = np.asarray(x, dtype=np.float32)
    Wq = np.asarray(Wq, dtype=np.float32)
    Wk = np.asarray(Wk, dtype=np.float32)
    Wv = np.asarray(Wv, dtype=np.float32)
    Wo = np.asarray(Wo, dtype=np.float32)

    if "nc" not in _CACHE:
        _CACHE["nc"] = _build()
    nc = _CACHE["nc"]

    from concourse.bass_utils import run_bass_kernel_spmd

    negI, vmask, vmaskP = _consts()
    woT = np.ascontiguousarray(Wo.T).astype(np.float16)
    in_maps = []
    for c in range(8):
        b, g = c // 2, c % 2
        sl = slice(512 * g, 512 * (g + 1))
        in_maps.append(
            {
                "xT": np.ascontiguousarray(x[b].T).astype(np.float16),
                "wqT": np.ascontiguousarray(Wq[sl, :].T).astype(np.float16),
                "wkT": np.ascontiguousarray(Wk[sl, :].T).astype(np.float16),
                "wvT": np.ascontiguousarray(Wv[sl, :].T).astype(np.float16),
                "woT": woT,
                "negI": negI,
                "vmask": vmask,
                "vmaskP": vmaskP,
            }
        )
    res = run_bass_kernel_spmd(nc, in_maps, list(range(8)))
    _CACHE["last_result"] = res
    y = np.empty((B, T, D), dtype=np.float32)
    for c in range(8):
        b, g = c // 2, c % 2
        y[b, 1024 * g : 1024 * (g + 1), :] = res.results[c]["out"]
    return y
